# revision 1
# baseline (speedup 1.0000x reference)
"""NT-Xent / SimCLR contrastive loss on 8 Trainium2 NeuronCores.

Problem: emb_i, emb_j [4096, 1024] f32 -> scalar loss.
  z = l2norm(rows); reps = concat(z_i, z_j) [8192, 1024]
  sim = reps @ reps.T;  loss = mean(-(pos/T - log(sum_offdiag exp(sim/T))))

Sharding (data parallel over the 8192 rows, 1024 rows per core):
  - each core normalizes its 1024 local rows, transposes them to [D, rows]
    (bf16), AllGathers the transposed normalized matrix,
  - computes its [1024, 8192] sim block with TensorE (bf16, f32 accum),
    fusing exp(2*sim) + row-sum into ScalarE activations (accum_out),
  - positives and the self-similarity diagonal are computed by a separate
    data-driven path (host supplies each core's partner row block), which
    keeps the single SPMD program free of core-dependent addressing,
  - per-row partial losses [128, 8] go back to the host, which sums and
    scales: a trivial gather.

Host-side work is only sharding/assembly: slicing rows, one np.eye, and a
final sum of the 8192 per-row loss terms.
"""

import numpy as np
import ml_dtypes

import concourse.bacc as bacc
import concourse.bass as bass
import concourse.mybir as mybir
import concourse.tile as tile
from concourse.bass_utils import run_bass_kernel_spmd

FP32 = mybir.dt.float32
BF16 = mybir.dt.bfloat16
AF = mybir.ActivationFunctionType
ALU = mybir.AluOpType

C = 8         # cores
N = 4096      # batch (per view)
D = 1024      # embedding dim
R = 1024      # local rows per core (2N / C)
P = 128       # partitions
MT = R // P   # m-tiles per core (8)
NT = 512      # matmul moving free dim (PSUM bank limit)
ESCALE = 2.0  # 1 / temperature


import os
_STAGE = int(os.environ.get("K_STAGE", "3"))


def _build_kernel(tc, nc, xloc, xpart, ident, out):
    with (
        tc.tile_pool(name="constp", bufs=1) as constp,
        tc.tile_pool(name="xmp", bufs=1) as xmp,      # 8 persistent local f32 tiles
        tc.tile_pool(name="zmp", bufs=1) as zmp,      # 8 persistent bf16 z tiles
        tc.tile_pool(name="ztp", bufs=1) as ztp,      # 8 persistent zT tiles
        tc.tile_pool(name="statp", bufs=1) as statp,
        tc.tile_pool(name="pp", bufs=3) as pp,        # partner row streaming
        tc.tile_pool(name="scrp", bufs=3) as scrp,    # [P, D] f32 scratch
        tc.tile_pool(name="gp", bufs=2) as gp,        # gathered tiles, 8 tags x 2
        tc.tile_pool(name="psp", bufs=4, space="PSUM") as psp,
        tc.tile_pool(name="ptp", bufs=2, space="PSUM") as ptp,
        tc.tile_pool(name="expp", bufs=4) as expp,
        tc.tile_pool(name="raccp", bufs=1) as raccp,
        tc.tile_pool(name="dramp", bufs=1, space="DRAM") as dramp,
    ):
        identt = constp.tile([P, P], BF16, name="identt")
        nc.sync.dma_start(identt[:], ident[:])

        ss = statp.tile([P, MT], FP32, name="ss")
        ssp = statp.tile([P, MT], FP32, name="ssp")
        ssz = statp.tile([P, MT], FP32, name="ssz")
        upos = statp.tile([P, MT], FP32, name="upos")

        # ---- phase 1: local row norms, scale, transpose ----
        xms = []
        for m in range(MT):
            xm = xmp.tile([P, D], FP32, name=f"xm{m}", tag=f"xm{m}")
            nc.sync.dma_start(xm[:], xloc[m * P:(m + 1) * P, :])
            sq = scrp.tile([P, D], FP32, name="sq", tag="scr")
            nc.scalar.activation(sq[:], xm[:], AF.Square,
                                 accum_out=ss[:, m:m + 1])
            xms.append(xm)

        # rs = 1/sqrt(ss) via exp(-0.5*ln(ss)) (Rsqrt ACT is banned; Ln+Exp
        # share one table set with Square and the main-loop Exp)
        lss = statp.tile([P, MT], FP32, name="lss")
        nc.scalar.activation(lss[:], ss[:], AF.Ln)
        rs = statp.tile([P, MT], FP32, name="rs")
        nc.scalar.activation(rs[:], lss[:], AF.Exp, scale=-0.5)

        zts = [ztp.tile([P, R], BF16, name=f"zt{d}", tag=f"zt{d}")
               for d in range(MT)]
        zms = []
        for m in range(MT):
            zm = zmp.tile([P, D], BF16, name=f"zm{m}", tag=f"zm{m}")
            nc.vector.tensor_scalar_mul(zm[:], xms[m][:], rs[:, m:m + 1])
            zms.append(zm)
            for d in range(8):
                pt = ptp.tile([P, P], BF16, name="pt", tag="pt")
                nc.tensor.transpose(pt[:], zm[:, d * P:(d + 1) * P], identt[:])
                nc.vector.tensor_copy(zts[d][:, m * P:(m + 1) * P], pt[:])

        if _STAGE < 2:
            nc.sync.dma_start(out[:], rs[:])
            return

        # ---- phase 2: AllGather the normalized transposed reps ----
        # The collective must run quiesced: concurrent DMA/engine activity
        # during a collective wedges this terminal's NRT (hang /
        # NRT_EXEC_UNIT_UNRECOVERABLE). Hence the explicit fences below.
        NCH = int(os.environ.get("K_AGCH", "1"))
        CR = R // NCH  # rows per chunk
        # NOTE: addr_space="Shared" outputs >~2 MiB wedge this terminal's
        # NRT (NRT_EXEC_UNIT_UNRECOVERABLE); Local outputs work at 16 MiB.
        ag_ins = [dramp.tile([R, CR], BF16, name=f"ag_in{k}")
                  for k in range(NCH)]
        ag_outs = [dramp.tile([C * R, CR], BF16, name=f"ag_out{k}")
                   for k in range(NCH)]
        asm_dmas = []
        for k in range(NCH):
            for d in range(8):
                asm_dmas.append(
                    nc.sync.dma_start(ag_ins[k][d * P:(d + 1) * P, :],
                                      zts[d][:, k * CR:(k + 1) * CR]))
        last_cc = None
        for k in range(NCH):
            cc = nc.gpsimd.collective_compute(
                "AllGather",
                ALU.bypass,
                replica_groups=[list(range(C))],
                ins=[ag_ins[k][:].opt()],
                outs=[ag_outs[k][:].opt()],
            )
            # Quiesce: no in-flight phase-1 DMAs while a collective runs.
            for dma in asm_dmas:
                tile.add_dep_helper(cc.ins, dma.ins,
                                    reason="collective after all asm DMAs")
            last_cc = cc

        if _STAGE == 15:  # phase1 + AG only
            probe = statp.tile([P, MT], FP32, name="probe")
            g0 = gp.tile([P, R], BF16, name="gprobe", tag="g0")
            for k in range(NCH):
                nc.sync.dma_start(g0[:, k * CR:(k + 1) * CR],
                                  ag_outs[k][56 * P:57 * P, :])
            nc.vector.tensor_copy(probe[:], g0[:, 0:MT])
            nc.sync.dma_start(out[:], probe[:])
            return

        # ---- phase 1b: partner norms + positives (after the collectives —
        # nothing may overlap the AG window, see above) ----
        _P1B = int(os.environ.get("K_P1B", "4"))
        for m in range(MT):
            pm = pp.tile([P, D], FP32, name="pm", tag="pm")
            pmd = nc.sync.dma_start(pm[:], xpart[m * P:(m + 1) * P, :])
            tile.add_dep_helper(pmd.ins, last_cc.ins,
                                reason="no DMA during collectives")
            sqp = scrp.tile([P, D], FP32, name="sqp", tag="scr")
            nc.scalar.activation(sqp[:], pm[:], AF.Square,
                                 accum_out=ssp[:, m:m + 1])
            if _P1B >= 2:
                # (tensor_tensor_reduce faults this terminal's NRT with an
                # INTERNAL error — use plain mul + reduce instead)
                um = scrp.tile([P, D], FP32, name="um", tag="scr")
                nc.vector.tensor_mul(um[:], xms[m][:], pm[:])
                nc.vector.reduce_sum(upos[:, m:m + 1], um[:],
                                     axis=mybir.AxisListType.X)
            if _P1B >= 3:
                sqz = scrp.tile([P, D], FP32, name="sqz", tag="scr")
                sqzi = nc.scalar.activation(sqz[:], zms[m][:], AF.Square,
                                            accum_out=ssz[:, m:m + 1])
                tile.add_dep_helper(sqzi.ins, last_cc.ins,
                                    reason="no engine work during collectives")
        if _P1B < 4 and _STAGE < 3:
            probe = statp.tile([P, MT], FP32, name="probe")
            g0 = gp.tile([P, R], BF16, name="gprobe", tag="g0")
            for k in range(NCH):
                nc.sync.dma_start(g0[:, k * CR:(k + 1) * CR],
                                  ag_outs[k][56 * P:57 * P, :])
            nc.vector.tensor_copy(probe[:], g0[:, 0:MT])
            nc.sync.dma_start(out[:], probe[:])
            return

        lssp = statp.tile([P, MT], FP32, name="lssp")
        nc.scalar.activation(lssp[:], ssp[:], AF.Ln)
        rsp = statp.tile([P, MT], FP32, name="rsp")
        nc.scalar.activation(rsp[:], lssp[:], AF.Exp, scale=-0.5)

        # pos2 = 2 * upos * rs * rsp
        t1 = statp.tile([P, MT], FP32, name="t1")
        nc.vector.tensor_mul(t1[:], upos[:], rs[:])
        t2 = statp.tile([P, MT], FP32, name="t2")
        nc.vector.tensor_mul(t2[:], t1[:], rsp[:])
        pos2 = statp.tile([P, MT], FP32, name="pos2")
        nc.vector.tensor_scalar_mul(pos2[:], t2[:], 2.0)

        # expself = exp(2 * ||z_bf16||^2) -- matches the diagonal term the
        # main matmul adds into each row-sum (same products, f32 accum)
        expself = statp.tile([P, MT], FP32, name="expself")
        nc.scalar.activation(expself[:], ssz[:], AF.Exp, scale=ESCALE)

        if _STAGE < 3:
            probe = statp.tile([P, MT], FP32, name="probe")
            g0 = gp.tile([P, R], BF16, name="gprobe", tag="g0")
            for k in range(NCH):
                nc.sync.dma_start(g0[:, k * CR:(k + 1) * CR],
                                  ag_outs[k][56 * P:57 * P, :])
            nc.vector.tensor_copy(probe[:], g0[:, 0:MT])
            nc.vector.tensor_sub(probe[:], probe[:], pos2[:])
            nc.sync.dma_start(out[:], probe[:])
            return

        # ---- phase 3: sim blocks + fused exp/row-sum ----
        raccs = [raccp.tile([P, 2 * C], FP32, name=f"racc{m}", tag=f"racc{m}")
                 for m in range(MT)]
        for cb in range(C):
            gs = []
            for d in range(8):
                g = gp.tile([P, R], BF16, name=f"g{d}", tag=f"g{d}")
                for k in range(NCH):
                    gd = nc.sync.dma_start(
                        g[:, k * CR:(k + 1) * CR],
                        ag_outs[k][(cb * 8 + d) * P:(cb * 8 + d + 1) * P, :])
                    tile.add_dep_helper(gd.ins, last_cc.ins,
                                        reason="no DMA during collectives")
                gs.append(g)
            for m in range(MT):
                ps0 = psp.tile([P, NT], FP32, name="ps0", tag="ps")
                ps1 = psp.tile([P, NT], FP32, name="ps1", tag="ps")
                for d in range(8):
                    lhs = zts[d][:, m * P:(m + 1) * P]
                    nc.tensor.matmul(ps0[:], lhs, gs[d][:, 0:NT],
                                     start=(d == 0), stop=(d == 7))
                    nc.tensor.matmul(ps1[:], lhs, gs[d][:, NT:2 * NT],
                                     start=(d == 0), stop=(d == 7))
                for nn, psx in ((0, ps0), (1, ps1)):
                    ed = expp.tile([P, NT], FP32, name="ed", tag="ed")
                    j = cb * 2 + nn
                    nc.scalar.activation(ed[:], psx[:], AF.Exp, scale=ESCALE,
                                         accum_out=raccs[m][:, j:j + 1])

        # ---- tail: denom, log, per-row loss ----
        rstot = statp.tile([P, MT], FP32, name="rstot")
        for m in range(MT):
            nc.vector.reduce_sum(rstot[:, m:m + 1], raccs[m][:],
                                 axis=mybir.AxisListType.X)
        denom = statp.tile([P, MT], FP32, name="denom")
        nc.vector.tensor_sub(denom[:], rstot[:], expself[:])
        logd = statp.tile([P, MT], FP32, name="logd")
        nc.scalar.activation(logd[:], denom[:], AF.Ln)
        outv = statp.tile([P, MT], FP32, name="outv")
        nc.vector.tensor_sub(outv[:], logd[:], pos2[:])
        nc.sync.dma_start(out[:], outv[:])


_NC_CACHE = {}


def build_nc():
    if "nc" in _NC_CACHE:
        return _NC_CACHE["nc"]
    nc = bacc.Bacc("TRN2", target_bir_lowering=False, debug=False,
                   num_devices=C)
    xloc = nc.dram_tensor("xloc", [R, D], FP32, kind="ExternalInput")
    xpart = nc.dram_tensor("xpart", [R, D], FP32, kind="ExternalInput")
    ident = nc.dram_tensor("ident", [P, P], BF16, kind="ExternalInput")
    out = nc.dram_tensor("out", [P, MT], FP32, kind="ExternalOutput")
    with tile.TileContext(nc) as tc:
        _build_kernel(tc, nc, xloc, xpart, ident, out)
    nc.compile()
    _NC_CACHE["nc"] = nc
    return nc


def run(emb_i, emb_j, **spmd_kwargs):
    x = np.concatenate(
        [np.asarray(emb_i, dtype=np.float32),
         np.asarray(emb_j, dtype=np.float32)], axis=0)
    eye = np.eye(P, dtype=ml_dtypes.bfloat16)
    in_maps = []
    for c in range(C):
        p = (c + C // 2) % C
        in_maps.append({
            "xloc": np.ascontiguousarray(x[c * R:(c + 1) * R]),
            "xpart": np.ascontiguousarray(x[p * R:(p + 1) * R]),
            "ident": eye,
        })
    nc = build_nc()
    res = run_bass_kernel_spmd(nc, in_maps, core_ids=list(range(C)),
                               **spmd_kwargs)
    total = np.float64(0.0)
    for c in range(C):
        total += np.asarray(res.results[c]["out"], dtype=np.float64).sum()
    loss = np.float32(total / (2 * N))
    return loss, res


def kernel(emb_i, emb_j):
    loss, _ = run(emb_i, emb_j)
    return np.asarray(loss, dtype=np.float32)



# revision 5
# speedup vs baseline: 1.5662x; 1.5662x over previous
"""NT-Xent / SimCLR contrastive loss on 8 Trainium2 NeuronCores.

Problem: emb_i, emb_j [4096, 1024] f32 -> scalar loss.
  z = l2norm(rows); reps = concat(z_i, z_j) [8192, 1024]
  sim = reps @ reps.T;  loss = mean(-(pos/T - log(sum_offdiag exp(sim/T))))

Sharding (data parallel over the 8192 rows, 1024 rows per core):
  - each core normalizes its 1024 local rows (scaled by S=64 so values sit
    in the fp8-e4m3 normal range), transposes them to [D, rows] and casts
    to fp8, AllGathers the transposed fp8 matrix (1 MiB per core),
  - computes its [1024, 8192] sim block with TensorE fp8 DoubleRow
    matmuls (2x contraction per pass, f32 accum), fusing
    exp(2*sim) = exp((2/S^2) * psum) + row-sum into one wide [128,1024]
    ScalarE activation per (block, m) pair,
  - positives and the self-similarity diagonal are computed by a separate
    f32 path (host supplies each core's partner row block), which keeps
    the single SPMD program free of core-dependent addressing,
  - per-row partial losses [128, 8] go back to the host, which sums and
    scales: a trivial gather.

Host-side work is only sharding/assembly: slicing rows, one np.eye, and a
final sum of the 8192 per-row loss terms.
"""

import math

import numpy as np
import ml_dtypes

import concourse.bacc as bacc
import concourse.bass as bass
import concourse.mybir as mybir
import concourse.tile as tile
from concourse.bass_utils import run_bass_kernel_spmd

FP32 = mybir.dt.float32
BF16 = mybir.dt.bfloat16
FP8 = mybir.dt.float8e4
AF = mybir.ActivationFunctionType
ALU = mybir.AluOpType
PM = mybir.MatmulPerfMode

C = 8         # cores
N = 4096      # batch (per view)
D = 1024      # embedding dim
R = 1024      # local rows per core (2N / C)
P = 128       # partitions
MT = R // P   # m-tiles per core (8)
NT = 512      # PSUM bank free size (f32)
ESCALE = 2.0  # 1 / temperature
S = 64.0      # fp8 pre-quantization scale; exp scale folds in 1/S^2
LNS = math.log(S)
QSCALE = ESCALE / (S * S)


def _build_kernel(tc, nc, xloc, xpart, ident, out):
    with (
        tc.tile_pool(name="constp", bufs=1) as constp,
        tc.tile_pool(name="xmp", bufs=1) as xmp,      # 8 persistent local f32 tiles
        tc.tile_pool(name="zmp", bufs=1) as zmp,      # 8 persistent bf16 z tiles
        tc.tile_pool(name="ztp", bufs=1) as ztp,      # one [P, MT, R] fp8 zT tile
        tc.tile_pool(name="statp", bufs=1) as statp,
        tc.tile_pool(name="pp", bufs=3) as pp,        # partner row streaming
        tc.tile_pool(name="scrp", bufs=3) as scrp,    # [P, D] discard scratch
        tc.tile_pool(name="gp", bufs=2) as gp,        # gathered fp8 tiles
        tc.tile_pool(name="psp", bufs=2, space="PSUM") as psp,   # [P,2*NT] = 2 banks each
        tc.tile_pool(name="ptp", bufs=2, space="PSUM") as ptp,
        tc.tile_pool(name="expp", bufs=4) as expp,
        tc.tile_pool(name="raccp", bufs=1) as raccp,
        tc.tile_pool(name="dramp", bufs=1, space="DRAM") as dramp,
    ):
        identt = constp.tile([P, P], BF16, name="identt")
        nc.sync.dma_start(identt[:], ident[:])

        lns = statp.tile([P, 1], FP32, name="lns")
        nc.vector.memset(lns[:], LNS)

        ss = statp.tile([P, MT], FP32, name="ss")
        ssp = statp.tile([P, MT], FP32, name="ssp")
        ssz = statp.tile([P, MT], FP32, name="ssz")
        upos = statp.tile([P, MT], FP32, name="upos")
        rs = statp.tile([P, MT], FP32, name="rs")
        lss = statp.tile([P, MT], FP32, name="lss")

        # ---- phase 1: local row norms, scale, transpose, fp8 cast ----
        # Per-m pipeline: dma -> square(accum) -> ln -> exp(-0.5*ln+ln(S))
        # -> scaled bf16 copy -> 8 PE transposes -> fp8 cast into zt.
        zt = ztp.tile([P, MT, R], FP8, name="zt")
        xms = []
        zms = []
        for m in range(MT):
            xm = xmp.tile([P, D], FP32, name=f"xm{m}", tag=f"xm{m}")
            nc.sync.dma_start(xm[:], xloc[m * P:(m + 1) * P, :])
            sq = scrp.tile([P, D], FP8, name="sq", tag="scr")
            nc.scalar.activation(sq[:], xm[:], AF.Square,
                                 accum_out=ss[:, m:m + 1])
            # rs = S/sqrt(ss) via exp(-0.5*ln(ss) + ln(S)) (Rsqrt ACT is
            # banned; Ln+Exp share one table set with Square and the
            # main-loop Exp)
            nc.scalar.activation(lss[:, m:m + 1], ss[:, m:m + 1], AF.Ln)
            nc.scalar.activation(rs[:, m:m + 1], lss[:, m:m + 1], AF.Exp,
                                 scale=-0.5, bias=lns[:])
            zm = zmp.tile([P, D], BF16, name=f"zm{m}", tag=f"zm{m}")
            nc.vector.tensor_scalar_mul(zm[:], xm[:], rs[:, m:m + 1])
            xms.append(xm)
            zms.append(zm)
            for d in range(8):
                pt = ptp.tile([P, P], BF16, name="pt", tag="pt")
                nc.tensor.transpose(pt[:], zm[:, d * P:(d + 1) * P], identt[:])
                nc.vector.tensor_copy(zt[:, d, m * P:(m + 1) * P], pt[:])

        # ---- phase 2: AllGather the normalized transposed fp8 reps ----
        # The collective must run quiesced: concurrent DMA/engine activity
        # during a collective wedges this terminal's NRT (hang /
        # NRT_EXEC_UNIT_UNRECOVERABLE). Hence the explicit fences below.
        # NOTE: addr_space="Shared" outputs >~2 MiB wedge this terminal's
        # NRT (NRT_EXEC_UNIT_UNRECOVERABLE); Local outputs work at 16 MiB.
        ag_in = dramp.tile([R, R], FP8, name="ag_in")
        ag_out = dramp.tile([C * R, R], FP8, name="ag_out")
        asm_dmas = []
        for d in range(8):
            asm_dmas.append(
                nc.sync.dma_start(ag_in[d * P:(d + 1) * P, :], zt[:, d, :]))
        cc = nc.gpsimd.collective_compute(
            "AllGather",
            ALU.bypass,
            replica_groups=[list(range(C))],
            ins=[ag_in[:].opt()],
            outs=[ag_out[:].opt()],
        )
        # Quiesce: no in-flight phase-1 DMAs while a collective runs.
        for dma in asm_dmas:
            tile.add_dep_helper(cc.ins, dma.ins,
                                reason="collective after all asm DMAs")
        last_cc = cc

        # ---- phase 1b: partner norms + positives (after the collectives —
        # nothing may overlap the AG window, see above) ----
        for m in range(MT):
            pm = pp.tile([P, D], FP32, name="pm", tag="pm")
            pmd = nc.sync.dma_start(pm[:], xpart[m * P:(m + 1) * P, :])
            tile.add_dep_helper(pmd.ins, last_cc.ins,
                                reason="no DMA during collectives")
            sqp = scrp.tile([P, D], FP8, name="sqp", tag="scr")
            nc.scalar.activation(sqp[:], pm[:], AF.Square,
                                 accum_out=ssp[:, m:m + 1])
            # (tensor_tensor_reduce faults this terminal's NRT with an
            # INTERNAL error — use plain mul + reduce instead)
            um = scrp.tile([P, D], FP32, name="um", tag="scr")
            nc.vector.tensor_mul(um[:], xms[m][:], pm[:])
            nc.vector.reduce_sum(upos[:, m:m + 1], um[:],
                                 axis=mybir.AxisListType.X)
            sqz = scrp.tile([P, D], FP8, name="sqz", tag="scr")
            sqzi = nc.scalar.activation(sqz[:], zms[m][:], AF.Square,
                                        accum_out=ssz[:, m:m + 1])
            tile.add_dep_helper(sqzi.ins, last_cc.ins,
                                reason="no engine work during collectives")

        lssp = statp.tile([P, MT], FP32, name="lssp")
        nc.scalar.activation(lssp[:], ssp[:], AF.Ln)
        rsp = statp.tile([P, MT], FP32, name="rsp")
        nc.scalar.activation(rsp[:], lssp[:], AF.Exp, scale=-0.5, bias=lns[:])

        # pos2 = 2 * upos * (S/||x||) * (S/||p||) / S^2
        t1 = statp.tile([P, MT], FP32, name="t1")
        nc.vector.tensor_mul(t1[:], upos[:], rs[:])
        t2 = statp.tile([P, MT], FP32, name="t2")
        nc.vector.tensor_mul(t2[:], t1[:], rsp[:])
        pos2 = statp.tile([P, MT], FP32, name="pos2")
        nc.vector.tensor_scalar_mul(pos2[:], t2[:], QSCALE)

        # expself = exp((2/S^2) * ||S*z_bf16||^2) -- matches the diagonal
        # term the fp8 matmul adds into each row-sum (up to fp8 rounding,
        # negligible vs the 8191-term denominator)
        expself = statp.tile([P, MT], FP32, name="expself")
        nc.scalar.activation(expself[:], ssz[:], AF.Exp, scale=QSCALE)

        # ---- phase 3: sim blocks + fused exp/row-sum ----
        # fp8 DoubleRow: each matmul consumes two 128-deep k-chunks.
        raccs = [raccp.tile([P, C], FP32, name=f"racc{m}", tag=f"racc{m}")
                 for m in range(MT)]
        for cb in range(C):
            g = gp.tile([P, MT, R], FP8, name="g", tag="g")
            for d in range(8):
                gd = nc.sync.dma_start(
                    g[:, d, :],
                    ag_out[(cb * 8 + d) * P:(cb * 8 + d + 1) * P, :])
                tile.add_dep_helper(gd.ins, last_cc.ins,
                                    reason="no DMA during collectives")
            for m in range(MT):
                ps = psp.tile([P, 2 * NT], FP32, name="ps", tag="ps")
                for k in range(4):
                    lhs = zt[:, 2 * k:2 * k + 2, m * P:(m + 1) * P]
                    nc.tensor.matmul(ps[:, 0:NT], lhs,
                                     g[:, 2 * k:2 * k + 2, 0:NT],
                                     start=(k == 0), stop=(k == 3),
                                     perf_mode=PM.DoubleRow)
                    nc.tensor.matmul(ps[:, NT:2 * NT], lhs,
                                     g[:, 2 * k:2 * k + 2, NT:2 * NT],
                                     start=(k == 0), stop=(k == 3),
                                     perf_mode=PM.DoubleRow)
                ed = expp.tile([P, 2 * NT], FP8, name="ed", tag="ed")
                nc.scalar.activation(ed[:], ps[:], AF.Exp, scale=QSCALE,
                                     accum_out=raccs[m][:, cb:cb + 1])

        # ---- tail: denom, log, per-row loss ----
        rstot = statp.tile([P, MT], FP32, name="rstot")
        for m in range(MT):
            nc.vector.reduce_sum(rstot[:, m:m + 1], raccs[m][:],
                                 axis=mybir.AxisListType.X)
        denom = statp.tile([P, MT], FP32, name="denom")
        nc.vector.tensor_sub(denom[:], rstot[:], expself[:])
        logd = statp.tile([P, MT], FP32, name="logd")
        nc.scalar.activation(logd[:], denom[:], AF.Ln)
        outv = statp.tile([P, MT], FP32, name="outv")
        nc.vector.tensor_sub(outv[:], logd[:], pos2[:])
        nc.sync.dma_start(out[:], outv[:])


_NC_CACHE = {}


def build_nc():
    if "nc" in _NC_CACHE:
        return _NC_CACHE["nc"]
    nc = bacc.Bacc("TRN2", target_bir_lowering=False, debug=False,
                   num_devices=C)
    xloc = nc.dram_tensor("xloc", [R, D], FP32, kind="ExternalInput")
    xpart = nc.dram_tensor("xpart", [R, D], FP32, kind="ExternalInput")
    ident = nc.dram_tensor("ident", [P, P], BF16, kind="ExternalInput")
    out = nc.dram_tensor("out", [P, MT], FP32, kind="ExternalOutput")
    with tile.TileContext(nc) as tc:
        _build_kernel(tc, nc, xloc, xpart, ident, out)
    nc.compile()
    _NC_CACHE["nc"] = nc
    return nc


def run(emb_i, emb_j, **spmd_kwargs):
    x = np.concatenate(
        [np.asarray(emb_i, dtype=np.float32),
         np.asarray(emb_j, dtype=np.float32)], axis=0)
    eye = np.eye(P, dtype=ml_dtypes.bfloat16)
    in_maps = []
    for c in range(C):
        p = (c + C // 2) % C
        in_maps.append({
            "xloc": np.ascontiguousarray(x[c * R:(c + 1) * R]),
            "xpart": np.ascontiguousarray(x[p * R:(p + 1) * R]),
            "ident": eye,
        })
    nc = build_nc()
    res = run_bass_kernel_spmd(nc, in_maps, core_ids=list(range(C)),
                               **spmd_kwargs)
    total = np.float64(0.0)
    for c in range(C):
        total += np.asarray(res.results[c]["out"], dtype=np.float64).sum()
    loss = np.float32(total / (2 * N))
    return loss, res


def kernel(emb_i, emb_j):
    loss, _ = run(emb_i, emb_j)
    return np.asarray(loss, dtype=np.float32)


# revision 10
# speedup vs baseline: 1.6204x; 1.0346x over previous
"""NT-Xent / SimCLR contrastive loss on 8 Trainium2 NeuronCores.

Problem: emb_i, emb_j [4096, 1024] f32 -> scalar loss.
  z = l2norm(rows); reps = concat(z_i, z_j) [2N, D]
  sim = reps @ reps.T;  loss = mean(-(pos/T - log(sum_offdiag exp(sim/T))))

Sharding (data parallel over the 2N=8192 rows, 1024 rows per core):
  - each core normalizes its 1024 local rows (scaled by S=64 so values sit
    in the fp8-e4m3 normal range), transposes them to [D, rows] fp8, and
    AllGathers the transposed fp8 matrix (1 MiB per core),
  - computes its [1024, 8192] sim block with TensorE fp8 DoubleRow
    matmuls (2x contraction per pass, f32 accum), fusing
    exp(2*sim) = exp((2/S^2) * psum) + row-sum into one wide [128,1024]
    ScalarE activation per (block, m) pair,
  - positives are computed by a separate f32 path (host supplies each
    core's partner row block), which keeps the single SPMD program free
    of core-dependent addressing; the self-similarity diagonal term is
    exp(2) to within fp8 rounding and is subtracted as a constant,
  - per-row partial losses [128, 8] go back to the host, which sums and
    scales: a trivial gather.

Host-side work is only sharding/assembly: slicing rows, one np.eye, and a
final sum of the 8192 per-row loss terms.
"""

import math

import numpy as np
import ml_dtypes

import concourse.bacc as bacc
import concourse.bass as bass
import concourse.mybir as mybir
import concourse.tile as tile
from concourse.bass_utils import run_bass_kernel_spmd

FP32 = mybir.dt.float32
BF16 = mybir.dt.bfloat16
FP8 = mybir.dt.float8e4
AF = mybir.ActivationFunctionType
ALU = mybir.AluOpType
PM = mybir.MatmulPerfMode

C = 8         # cores
N = 4096      # batch (per view)
D = 1024      # embedding dim
R = 1024      # local rows per core (2N / C)
P = 128       # partitions
MT = R // P   # m-tiles per core (8)
NT = 512      # PSUM bank free size (f32)
ESCALE = 2.0  # 1 / temperature
S = 64.0      # fp8 pre-quantization scale; exp scale folds in 1/S^2
LNS = math.log(S)
QSCALE = ESCALE / (S * S)
EDIAG = math.exp(ESCALE)  # self-sim diagonal term, exact to fp8 rounding


def _build_kernel(tc, nc, xloc, xpart, ident, out):
    with (
        tc.tile_pool(name="constp", bufs=1) as constp,
        tc.tile_pool(name="xmp", bufs=1) as xmp,      # 8 persistent local f32 tiles
        tc.tile_pool(name="pmp", bufs=1) as pmp,      # 8 persistent partner f32 tiles
        tc.tile_pool(name="zmp", bufs=1) as zmp,      # 8 persistent fp8 z tiles
        tc.tile_pool(name="ztp", bufs=1) as ztp,      # one [P, MT, R] fp8 zT tile
        tc.tile_pool(name="statp", bufs=1) as statp,
        tc.tile_pool(name="scrp", bufs=3) as scrp,    # [P, D] discard scratch
        tc.tile_pool(name="gp", bufs=2) as gp,        # gathered fp8 tiles
        tc.tile_pool(name="psp", bufs=2, space="PSUM") as psp,   # [P,2*NT] = 2 banks each
        tc.tile_pool(name="ptp", bufs=2, space="PSUM") as ptp,   # fp8 transpose staging
        tc.tile_pool(name="expp", bufs=4) as expp,
        tc.tile_pool(name="raccp", bufs=1) as raccp,
        tc.tile_pool(name="dramp", bufs=1, space="DRAM") as dramp,
    ):
        identt = constp.tile([P, P], BF16, name="identt")
        nc.sync.dma_start(identt[:], ident[:])

        lns = statp.tile([P, 1], FP32, name="lns")
        nc.vector.memset(lns[:], LNS)

        ss = statp.tile([P, MT], FP32, name="ss")
        ssp = statp.tile([P, MT], FP32, name="ssp")
        upos = statp.tile([P, MT], FP32, name="upos")
        rs = statp.tile([P, MT], FP32, name="rs")
        lss = statp.tile([P, MT], FP32, name="lss")

        # ---- phase 1: local+partner row norms, scale, transpose to fp8 ----
        # All ScalarE Squares are grouped (one act-table load), then a
        # single Ln+Exp pair computes rs = S/||x|| = exp(-.5*ln(ss)+ln(S))
        # for all 8 row tiles at once (Rsqrt ACT is banned).
        xms = []
        pms = []
        for m in range(MT):
            xm = xmp.tile([P, D], FP32, name=f"xm{m}", tag=f"xm{m}")
            nc.sync.dma_start(xm[:], xloc[m * P:(m + 1) * P, :])
            sq = scrp.tile([P, D], FP8, name="sq", tag="scr")
            nc.scalar.activation(sq[:], xm[:], AF.Square,
                                 accum_out=ss[:, m:m + 1])
            xms.append(xm)
        nc.scalar.activation(lss[:], ss[:], AF.Ln)
        nc.scalar.activation(rs[:], lss[:], AF.Exp, scale=-0.5, bias=lns[:])
        # partner norms (squares only; Ln/Exp for them runs post-AG)
        for m in range(MT):
            pm = pmp.tile([P, D], FP32, name=f"pm{m}", tag=f"pm{m}")
            nc.sync.dma_start(pm[:], xpart[m * P:(m + 1) * P, :])
            sqp = scrp.tile([P, D], FP8, name="sqp", tag="scr")
            nc.scalar.activation(sqp[:], pm[:], AF.Square,
                                 accum_out=ssp[:, m:m + 1])
            pms.append(pm)

        zt = ztp.tile([P, MT, R], FP8, name="zt")
        for m in range(MT):
            zm = zmp.tile([P, D], BF16, name=f"zm{m}", tag=f"zm{m}")
            nc.vector.tensor_scalar_mul(zm[:], xms[m][:], rs[:, m:m + 1])
            pt = ptp.tile([P, MT, P], BF16, name="pt", tag="pt")
            for d in range(8):
                nc.tensor.matmul(pt[:, d, :], zm[:, d * P:(d + 1) * P],
                                 identt[:], is_transpose=True,
                                 skip_group_check=True)
            nc.vector.tensor_copy(zt[:, :, m * P:(m + 1) * P], pt[:])

        # ---- phase 2: AllGather the normalized transposed fp8 reps ----
        # The collective must run quiesced: concurrent DMA/engine activity
        # during a collective wedges this terminal's NRT (hang /
        # NRT_EXEC_UNIT_UNRECOVERABLE). Hence the explicit fences below.
        # NOTE: addr_space="Shared" outputs >~2 MiB wedge this terminal's
        # NRT (NRT_EXEC_UNIT_UNRECOVERABLE); Local outputs work at 16 MiB.
        ag_in = dramp.tile([R, R], FP8, name="ag_in")
        ag_out = dramp.tile([C * R, R], FP8, name="ag_out")
        asm_dmas = []
        for d in range(8):
            asm_dmas.append(
                nc.sync.dma_start(ag_in[d * P:(d + 1) * P, :], zt[:, d, :]))
        cc = nc.gpsimd.collective_compute(
            "AllGather",
            ALU.bypass,
            replica_groups=[list(range(C))],
            ins=[ag_in[:].opt()],
            outs=[ag_out[:].opt()],
        )
        # Quiesce: no in-flight phase-1 DMAs while a collective runs.
        for dma in asm_dmas:
            tile.add_dep_helper(cc.ins, dma.ins,
                                reason="collective after all asm DMAs")
        last_cc = cc

        # ---- phase 1b: positives (after the collective — nothing may
        # overlap the AG window, see above; runs on DVE/ScalarE while
        # TensorE streams phase-3 matmuls) ----
        lssp = statp.tile([P, MT], FP32, name="lssp")
        li = nc.scalar.activation(lssp[:], ssp[:], AF.Ln)
        tile.add_dep_helper(li.ins, last_cc.ins,
                            reason="no engine work during collectives")
        rsp = statp.tile([P, MT], FP32, name="rsp")
        nc.scalar.activation(rsp[:], lssp[:], AF.Exp, scale=-0.5, bias=lns[:])
        for m in range(MT):
            # (tensor_tensor_reduce faults this terminal's NRT with an
            # INTERNAL error — use plain mul + reduce instead)
            um = scrp.tile([P, D], FP32, name="um", tag="scr")
            umi = nc.vector.tensor_mul(um[:], xms[m][:], pms[m][:])
            tile.add_dep_helper(umi.ins, last_cc.ins,
                                reason="no engine work during collectives")
            nc.vector.reduce_sum(upos[:, m:m + 1], um[:],
                                 axis=mybir.AxisListType.X)

        # pos2 = 2 * upos * (S/||x||) * (S/||p||) / S^2
        t1 = statp.tile([P, MT], FP32, name="t1")
        nc.vector.tensor_mul(t1[:], upos[:], rs[:])
        t2 = statp.tile([P, MT], FP32, name="t2")
        nc.vector.tensor_mul(t2[:], t1[:], rsp[:])
        pos2 = statp.tile([P, MT], FP32, name="pos2")
        nc.vector.tensor_scalar_mul(pos2[:], t2[:], QSCALE)

        # ---- phase 3: sim blocks + fused exp/row-sum ----
        # fp8 DoubleRow: each matmul consumes two 128-deep k-chunks.
        raccs = [raccp.tile([P, C], FP32, name=f"racc{m}", tag=f"racc{m}")
                 for m in range(MT)]
        for cb in range(C):
            g = gp.tile([P, MT, R], FP8, name="g", tag="g")
            for d in range(8):
                gd = nc.sync.dma_start(
                    g[:, d, :],
                    ag_out[(cb * 8 + d) * P:(cb * 8 + d + 1) * P, :])
                tile.add_dep_helper(gd.ins, last_cc.ins,
                                    reason="no DMA during collectives")
            for m in range(MT):
                ps = psp.tile([P, 2 * NT], FP32, name="ps", tag="ps")
                for k in range(4):
                    lhs = zt[:, 2 * k:2 * k + 2, m * P:(m + 1) * P]
                    nc.tensor.matmul(ps[:, 0:NT], lhs,
                                     g[:, 2 * k:2 * k + 2, 0:NT],
                                     start=(k == 0), stop=(k == 3),
                                     perf_mode=PM.DoubleRow)
                    nc.tensor.matmul(ps[:, NT:2 * NT], lhs,
                                     g[:, 2 * k:2 * k + 2, NT:2 * NT],
                                     start=(k == 0), stop=(k == 3),
                                     perf_mode=PM.DoubleRow)
                ed = expp.tile([P, 2 * NT], FP8, name="ed", tag="ed")
                nc.scalar.activation(ed[:], ps[:], AF.Exp, scale=QSCALE,
                                     accum_out=raccs[m][:, cb:cb + 1])

        # ---- tail: denom, log, per-row loss ----
        rstot = statp.tile([P, MT], FP32, name="rstot")
        for m in range(MT):
            nc.vector.reduce_sum(rstot[:, m:m + 1], raccs[m][:],
                                 axis=mybir.AxisListType.X)
        denom = statp.tile([P, MT], FP32, name="denom")
        nc.vector.tensor_scalar_sub(denom[:], rstot[:], EDIAG)
        logd = statp.tile([P, MT], FP32, name="logd")
        nc.scalar.activation(logd[:], denom[:], AF.Ln)
        outv = statp.tile([P, MT], FP32, name="outv")
        nc.vector.tensor_sub(outv[:], logd[:], pos2[:])
        nc.sync.dma_start(out[:], outv[:])


_NC_CACHE = {}


def build_nc():
    if "nc" in _NC_CACHE:
        return _NC_CACHE["nc"]
    nc = bacc.Bacc("TRN2", target_bir_lowering=False, debug=False,
                   num_devices=C)
    xloc = nc.dram_tensor("xloc", [R, D], FP32, kind="ExternalInput")
    xpart = nc.dram_tensor("xpart", [R, D], FP32, kind="ExternalInput")
    ident = nc.dram_tensor("ident", [P, P], BF16, kind="ExternalInput")
    out = nc.dram_tensor("out", [P, MT], FP32, kind="ExternalOutput")
    with tile.TileContext(nc) as tc:
        _build_kernel(tc, nc, xloc, xpart, ident, out)
    nc.compile()
    _NC_CACHE["nc"] = nc
    return nc


def run(emb_i, emb_j, **spmd_kwargs):
    x = np.concatenate(
        [np.asarray(emb_i, dtype=np.float32),
         np.asarray(emb_j, dtype=np.float32)], axis=0)
    eye = np.eye(P, dtype=ml_dtypes.bfloat16)
    in_maps = []
    for c in range(C):
        p = (c + C // 2) % C
        in_maps.append({
            "xloc": np.ascontiguousarray(x[c * R:(c + 1) * R]),
            "xpart": np.ascontiguousarray(x[p * R:(p + 1) * R]),
            "ident": eye,
        })
    nc = build_nc()
    res = run_bass_kernel_spmd(nc, in_maps, core_ids=list(range(C)),
                               **spmd_kwargs)
    total = np.float64(0.0)
    for c in range(C):
        total += np.asarray(res.results[c]["out"], dtype=np.float64).sum()
    loss = np.float32(total / (2 * N))
    return loss, res


def kernel(emb_i, emb_j):
    loss, _ = run(emb_i, emb_j)
    return np.asarray(loss, dtype=np.float32)


# revision 13
# speedup vs baseline: 1.7769x; 1.0966x over previous
"""NT-Xent / SimCLR contrastive loss on 8 Trainium2 NeuronCores.

Problem: emb_i, emb_j [4096, 1024] f32 -> scalar loss.
  z = l2norm(rows); reps = concat(z_i, z_j) [2N, D]
  sim = reps @ reps.T;  loss = mean(-(pos/T - log(sum_offdiag exp(sim/T))))

Sharding (data parallel over the 2N=8192 rows, 1024 rows per core):
  - each core normalizes its 1024 local rows (scaled by S=64 so values sit
    in the fp8-e4m3 normal range), transposes them to [D, rows] fp8, and
    AllGathers the transposed fp8 matrix (1 MiB per core),
  - computes its [1024, 8192] sim block with TensorE fp8 DoubleRow
    matmuls (2x contraction per pass, f32 accum), fusing
    exp(2*sim) = exp((2/S^2) * psum) + row-sum into one wide [128,1024]
    ScalarE activation per (block, m) pair,
  - positives are computed by a separate f32 path (host supplies each
    core's partner row block), which keeps the single SPMD program free
    of core-dependent addressing; the self-similarity diagonal term is
    exp(2) to within fp8 rounding and is subtracted as a constant,
  - per-row partial losses [128, 8] go back to the host, which sums and
    scales: a trivial gather.

Host-side work is only sharding/assembly: slicing rows, one np.eye, and a
final sum of the 8192 per-row loss terms.
"""

import math

import numpy as np
import ml_dtypes

import concourse.bacc as bacc
import concourse.bass as bass
import concourse.mybir as mybir
import concourse.tile as tile
from concourse.bass_utils import run_bass_kernel_spmd

FP32 = mybir.dt.float32
BF16 = mybir.dt.bfloat16
FP8 = mybir.dt.float8e4
AF = mybir.ActivationFunctionType
ALU = mybir.AluOpType
PM = mybir.MatmulPerfMode

C = 8         # cores
N = 4096      # batch (per view)
D = 1024      # embedding dim
R = 1024      # local rows per core (2N / C)
P = 128       # partitions
MT = R // P   # m-tiles per core (8)
NT = 512      # PSUM bank free size (f32)
ESCALE = 2.0  # 1 / temperature
S = 64.0      # fp8 pre-quantization scale; exp scale folds in 1/S^2
LNS = math.log(S)
QSCALE = ESCALE / (S * S)
EDIAG = math.exp(ESCALE)  # self-sim diagonal term, exact to fp8 rounding


def _build_kernel(tc, nc, xloc, xpart, ident, out):
    with (
        tc.tile_pool(name="constp", bufs=1) as constp,
        tc.tile_pool(name="xmp", bufs=1) as xmp,      # 8 persistent local f32 tiles
        tc.tile_pool(name="pmp", bufs=1) as pmp,      # 8 persistent partner f32 tiles
        tc.tile_pool(name="zmp", bufs=1) as zmp,      # 8 persistent fp8 z tiles
        tc.tile_pool(name="ztp", bufs=1) as ztp,      # one [P, MT, R] fp8 zT tile
        tc.tile_pool(name="statp", bufs=1) as statp,
        tc.tile_pool(name="scrp", bufs=3) as scrp,    # [P, D] discard scratch
        tc.tile_pool(name="gp", bufs=2) as gp,        # gathered fp8 tiles
        tc.tile_pool(name="psp", bufs=3, space="PSUM") as psp,   # [P,2*NT] = 2 banks each
        tc.tile_pool(name="ptp", bufs=2, space="PSUM") as ptp,   # fp8 transpose staging
        tc.tile_pool(name="expp", bufs=4) as expp,
        tc.tile_pool(name="raccp", bufs=1) as raccp,
        tc.tile_pool(name="dramp", bufs=1, space="DRAM") as dramp,
    ):
        identt = constp.tile([P, P], BF16, name="identt")
        nc.sync.dma_start(identt[:], ident[:])

        lns = statp.tile([P, 1], FP32, name="lns")
        nc.vector.memset(lns[:], LNS)

        ss = statp.tile([P, MT], FP32, name="ss")
        ssp = statp.tile([P, MT], FP32, name="ssp")
        upos = statp.tile([P, MT], FP32, name="upos")
        rs = statp.tile([P, MT], FP32, name="rs")
        lss = statp.tile([P, MT], FP32, name="lss")

        # ---- phase 1: local+partner row norms, scale, transpose to fp8 ----
        # All ScalarE Squares are grouped (one act-table load), then a
        # single Ln+Exp pair computes rs = S/||x|| = exp(-.5*ln(ss)+ln(S))
        # for all 8 row tiles at once (Rsqrt ACT is banned).
        xms = []
        pms = []
        for m in range(MT):
            xm = xmp.tile([P, D], FP32, name=f"xm{m}", tag=f"xm{m}")
            nc.sync.dma_start(xm[:], xloc[m * P:(m + 1) * P, :])
            sq = scrp.tile([P, D], FP8, name="sq", tag="scr")
            nc.scalar.activation(sq[:], xm[:], AF.Square,
                                 accum_out=ss[:, m:m + 1])
            xms.append(xm)
        nc.scalar.activation(lss[:], ss[:], AF.Ln)
        nc.scalar.activation(rs[:], lss[:], AF.Exp, scale=-0.5, bias=lns[:])
        # partner norms (squares only; Ln/Exp for them runs post-AG)
        for m in range(MT):
            pm = pmp.tile([P, D], FP32, name=f"pm{m}", tag=f"pm{m}")
            nc.sync.dma_start(pm[:], xpart[m * P:(m + 1) * P, :])
            sqp = scrp.tile([P, D], FP8, name="sqp", tag="scr")
            nc.scalar.activation(sqp[:], pm[:], AF.Square,
                                 accum_out=ssp[:, m:m + 1])
            pms.append(pm)

        zt = ztp.tile([P, MT, R], FP8, name="zt")
        for m in range(MT):
            zm = zmp.tile([P, D], BF16, name=f"zm{m}", tag=f"zm{m}")
            nc.vector.tensor_scalar_mul(zm[:], xms[m][:], rs[:, m:m + 1])
            pt = ptp.tile([P, MT, P], BF16, name="pt", tag="pt")
            for d in range(8):
                nc.tensor.matmul(pt[:, d, :], zm[:, d * P:(d + 1) * P],
                                 identt[:], is_transpose=True,
                                 skip_group_check=True)
            nc.vector.tensor_copy(zt[:, :, m * P:(m + 1) * P], pt[:])

        # ---- phase 2: AllGather the normalized transposed fp8 reps ----
        # The collective must run quiesced: concurrent DMA/engine activity
        # during a collective wedges this terminal's NRT (hang /
        # NRT_EXEC_UNIT_UNRECOVERABLE). Hence the explicit fences below.
        # NOTE: addr_space="Shared" outputs >~2 MiB wedge this terminal's
        # NRT (NRT_EXEC_UNIT_UNRECOVERABLE); Local outputs work at 16 MiB.
        ag_in = dramp.tile([R, R], FP8, name="ag_in")
        ag_out = dramp.tile([C * R, R], FP8, name="ag_out")
        asm_dmas = []
        for d in range(8):
            asm_dmas.append(
                nc.sync.dma_start(ag_in[d * P:(d + 1) * P, :], zt[:, d, :]))
        cc = nc.gpsimd.collective_compute(
            "AllGather",
            ALU.bypass,
            replica_groups=[list(range(C))],
            ins=[ag_in[:].opt()],
            outs=[ag_out[:].opt()],
        )
        # Quiesce: no in-flight phase-1 DMAs while a collective runs.
        for dma in asm_dmas:
            tile.add_dep_helper(cc.ins, dma.ins,
                                reason="collective after all asm DMAs")
        last_cc = cc

        # ---- phase 1b: positives (after the collective — nothing may
        # overlap the AG window, see above; runs on DVE/ScalarE while
        # TensorE streams phase-3 matmuls) ----
        lssp = statp.tile([P, MT], FP32, name="lssp")
        li = nc.scalar.activation(lssp[:], ssp[:], AF.Ln)
        tile.add_dep_helper(li.ins, last_cc.ins,
                            reason="no engine work during collectives")
        rsp = statp.tile([P, MT], FP32, name="rsp")
        nc.scalar.activation(rsp[:], lssp[:], AF.Exp, scale=-0.5, bias=lns[:])
        for m in range(MT):
            # (tensor_tensor_reduce faults this terminal's NRT with an
            # INTERNAL error — use plain mul + reduce instead)
            um = scrp.tile([P, D], FP32, name="um", tag="scr")
            umi = nc.vector.tensor_mul(um[:], xms[m][:], pms[m][:])
            tile.add_dep_helper(umi.ins, last_cc.ins,
                                reason="no engine work during collectives")
            nc.vector.reduce_sum(upos[:, m:m + 1], um[:],
                                 axis=mybir.AxisListType.X)

        # pos2 = 2 * upos * (S/||x||) * (S/||p||) / S^2
        t1 = statp.tile([P, MT], FP32, name="t1")
        nc.vector.tensor_mul(t1[:], upos[:], rs[:])
        t2 = statp.tile([P, MT], FP32, name="t2")
        nc.vector.tensor_mul(t2[:], t1[:], rsp[:])
        pos2 = statp.tile([P, MT], FP32, name="pos2")
        nc.vector.tensor_scalar_mul(pos2[:], t2[:], QSCALE)

        # ---- phase 3: sim blocks + fused exp/row-sum ----
        # fp8 DoubleRow: each matmul consumes two 128-deep k-chunks.
        racc = raccp.tile([P, MT, C], FP32, name="racc")
        for cb in range(C):
            g = gp.tile([P, MT, R], FP8, name="g", tag="g")
            for d in range(8):
                gd = nc.sync.dma_start(
                    g[:, d, :],
                    ag_out[(cb * 8 + d) * P:(cb * 8 + d + 1) * P, :])
                tile.add_dep_helper(gd.ins, last_cc.ins,
                                    reason="no DMA during collectives")
            for m in range(MT):
                ps = psp.tile([P, 2 * NT], FP32, name="ps", tag="ps")
                for k in range(4):
                    lhs = zt[:, 2 * k:2 * k + 2, m * P:(m + 1) * P]
                    nc.tensor.matmul(ps[:, 0:NT], lhs,
                                     g[:, 2 * k:2 * k + 2, 0:NT],
                                     start=(k == 0), stop=(k == 3),
                                     perf_mode=PM.DoubleRow)
                    nc.tensor.matmul(ps[:, NT:2 * NT], lhs,
                                     g[:, 2 * k:2 * k + 2, NT:2 * NT],
                                     start=(k == 0), stop=(k == 3),
                                     perf_mode=PM.DoubleRow)
                ed = expp.tile([P, 2 * NT], FP8, name="ed", tag="ed")
                nc.scalar.activation(ed[:], ps[:], AF.Exp, scale=QSCALE,
                                     accum_out=racc[:, m, cb:cb + 1])

        # ---- tail: denom, log, per-row loss ----
        rstot = statp.tile([P, MT], FP32, name="rstot")
        nc.vector.reduce_sum(rstot[:], racc[:], axis=mybir.AxisListType.X)
        denom = statp.tile([P, MT], FP32, name="denom")
        nc.vector.tensor_scalar_sub(denom[:], rstot[:], EDIAG)
        logd = statp.tile([P, MT], FP32, name="logd")
        nc.scalar.activation(logd[:], denom[:], AF.Ln)
        outv = statp.tile([P, MT], FP32, name="outv")
        nc.vector.tensor_sub(outv[:], logd[:], pos2[:])
        nc.sync.dma_start(out[:], outv[:])


_NC_CACHE = {}


def build_nc():
    if "nc" in _NC_CACHE:
        return _NC_CACHE["nc"]
    nc = bacc.Bacc("TRN2", target_bir_lowering=False, debug=False,
                   num_devices=C)
    xloc = nc.dram_tensor("xloc", [R, D], FP32, kind="ExternalInput")
    xpart = nc.dram_tensor("xpart", [R, D], FP32, kind="ExternalInput")
    ident = nc.dram_tensor("ident", [P, P], BF16, kind="ExternalInput")
    out = nc.dram_tensor("out", [P, MT], FP32, kind="ExternalOutput")
    with tile.TileContext(nc) as tc:
        _build_kernel(tc, nc, xloc, xpart, ident, out)
    nc.compile()
    _NC_CACHE["nc"] = nc
    return nc


def run(emb_i, emb_j, **spmd_kwargs):
    x = np.concatenate(
        [np.asarray(emb_i, dtype=np.float32),
         np.asarray(emb_j, dtype=np.float32)], axis=0)
    eye = np.eye(P, dtype=ml_dtypes.bfloat16)
    in_maps = []
    for c in range(C):
        p = (c + C // 2) % C
        in_maps.append({
            "xloc": np.ascontiguousarray(x[c * R:(c + 1) * R]),
            "xpart": np.ascontiguousarray(x[p * R:(p + 1) * R]),
            "ident": eye,
        })
    nc = build_nc()
    res = run_bass_kernel_spmd(nc, in_maps, core_ids=list(range(C)),
                               **spmd_kwargs)
    total = np.float64(0.0)
    for c in range(C):
        total += np.asarray(res.results[c]["out"], dtype=np.float64).sum()
    loss = np.float32(total / (2 * N))
    return loss, res


def kernel(emb_i, emb_j):
    loss, _ = run(emb_i, emb_j)
    return np.asarray(loss, dtype=np.float32)
